# revision 8
# baseline (speedup 1.0000x reference)
"""Trainium2 Bass kernel for nn_STSourceModule (segment_reduce).

Math: source_ids x are binary {0,1}, so the masked softmax over sites
collapses to a closed form.  With g[n] = exp(fire_bias[n]),
A0[h] = exp(attn_b[h]), A1[h] = exp(attn_b[h] + attn_w[h]):

  Z[s,h,c]   = A0[h]*(S0[c] - T1[s,c]) + A1[h]*T1[s,c]
  r[s,c,h]   = A1[h]*T1[s,c] / Z[s,h,c]
  out[s,c,:] = mask[c]*base + sum_h r[s,c,h]*Wh[h,:]

where S0[c] = sum_{n in c} g[n], T1[s,c] = sum_{n in c} x[s,n]*g[n],
base = ffn_b + ffn_w@val_b, Wh[h] = ffn_w[:,32h:32h+32]@val_w[32h:32h+32].

Sharding: data-parallel over batch B=8, one batch element per core.

The rank-5 expansion (the only large-output stage) is split between the
PE (matmul per cluster, c < PE_C) and the DVE (chained tensor_scalar FMA
per cluster, c >= PE_C) so both engines work in parallel; ACT drains
PSUM for the PE tiles.
"""

import sys

for _p in ("/opt/trn_rl_repo",):
    if _p not in sys.path:
        sys.path.insert(0, _p)

from contextlib import ExitStack

import numpy as np

import concourse.bass as bass
import concourse.tile as tile
from concourse import bacc, mybir
from concourse.bass_utils import run_bass_kernel_spmd
from concourse.masks import make_identity

F32 = mybir.dt.float32
AF = mybir.ActivationFunctionType
ALU = mybir.AluOpType

MAX_SP, MAX_TP = 180.0, 365.0
B, S, N, C = 8, 256, 128, 64
NH, HID, FH = 4, 256, 32

PE_C = 40               # clusters expanded on the PE; rest on the DVE

TRACE = False           # set True (e.g. from test.py) to neuron-profile
LAST_RESULT = None      # BassKernelResults of the last run


def _build_program(csp, ctp, a0, a1):
    nc = bacc.Bacc()

    x_d = nc.declare_dram_parameter("x", [S, N], F32, isOutput=False)
    nv_d = nc.declare_dram_parameter("nv", [N, 4], F32, isOutput=False)
    cv_d = nc.declare_dram_parameter("cv", [1, 192], F32, isOutput=False)
    waug_d = nc.declare_dram_parameter("waug", [5, HID], F32, isOutput=False)
    wf_d = nc.declare_dram_parameter("wf", [1, 5 * HID], F32, isOutput=False)
    out_d = nc.declare_dram_parameter("out", [S, C, HID], F32, isOutput=True)

    da = [float(a1[h] - a0[h]) for h in range(NH)]

    with tile.TileContext(nc) as tc, ExitStack() as ctx:
        consts = ctx.enter_context(tc.tile_pool(name="consts", bufs=1))
        work = ctx.enter_context(tc.tile_pool(name="work", bufs=2))
        tpsum = ctx.enter_context(tc.tile_pool(name="tpsum", bufs=2, space="PSUM"))
        opsum = ctx.enter_context(tc.tile_pool(name="opsum", bufs=4, space="PSUM"))
        outp = ctx.enter_context(tc.tile_pool(name="outp", bufs=4))

        # ---- inputs (one DMA each) -------------------------------------
        ident = consts.tile([128, 128], F32)
        make_identity(nc, ident)

        cv = consts.tile([128, 192], F32)
        nc.sync.dma_start(out=cv, in_=cv_d[:, :].to_broadcast([128, 192]))
        wf = consts.tile([128, 5 * HID], F32)
        nc.sync.dma_start(out=wf, in_=wf_d[:, :].to_broadcast([128, 5 * HID]))
        waug = consts.tile([5, HID], F32)
        nc.sync.dma_start(out=waug, in_=waug_d[:, :])
        nv = consts.tile([128, 4], F32)
        nc.sync.dma_start(out=nv, in_=nv_d[:, :])
        xf = consts.tile([128, 2, N], F32)
        nc.sync.dma_start(out=xf, in_=x_d[:, :].rearrange("(q p) n -> p q n", p=128))

        ones_col = consts.tile([128, 1], F32)
        nc.vector.memset(ones_col, 1.0)
        ones_row = consts.tile([1, 128], F32)
        nc.vector.memset(ones_row, 1.0)

        # cv layout: [iota(64) | w1s(32) w1t(32) | w2s(32) w2t(32)]
        iot, w1cat, w2cat = cv[:, 0:64], cv[:, 64:128], cv[:, 128:192]
        labs, dsp, dtp = nv[:, 0:1], nv[:, 1:2], nv[:, 2:3]

        # ---- FIRE bias -> g = exp(bias) --------------------------------
        dls = work.tile([128, 1], F32)
        nc.scalar.activation(out=dls, in_=dsp, func=AF.Ln, bias=1.0, scale=csp)
        dlt = work.tile([128, 1], F32)
        nc.scalar.activation(out=dlt, in_=dtp, func=AF.Ln, bias=1.0, scale=ctp)
        h = work.tile([128, 2 * FH], F32)
        nc.vector.tensor_scalar_mul(out=h[:, 0:FH], in0=w1cat[:, 0:FH], scalar1=dls)
        nc.vector.tensor_scalar_mul(out=h[:, FH:], in0=w1cat[:, FH:], scalar1=dlt)
        nc.scalar.activation(out=h, in_=h, func=AF.Silu)
        nc.vector.tensor_mul(out=h, in0=h, in1=w2cat)
        bsum = work.tile([128, 1], F32)
        nc.vector.reduce_sum(out=bsum, in_=h, axis=mybir.AxisListType.X)
        g = work.tile([128, 1], F32)
        nc.scalar.activation(out=g, in_=bsum, func=AF.Exp)

        # ---- mg[n,c] = (lab[n]==c) * g[n] ------------------------------
        mg = work.tile([128, C], F32)
        nc.vector.tensor_scalar(
            out=mg, in0=iot, scalar1=labs, scalar2=g,
            op0=ALU.is_equal, op1=ALU.mult,
        )

        # ---- xT via PE transpose (dummy first: lone-LDWEIGHTS sync) ----
        ptd = tpsum.tile([128, 128], F32, tag="pt")
        nc.tensor.transpose(ptd, ident, ident)
        xT = work.tile([128, S], F32)
        for i in range(2):
            pt = tpsum.tile([128, 128], F32, tag="pt")
            nc.tensor.transpose(pt, xf[:, i, :], ident)
            nc.scalar.copy(out=xT[:, i * 128:(i + 1) * 128], in_=pt)

        # ---- segment sums ----------------------------------------------
        t1_ps = tpsum.tile([64, S], F32, tag="pt")
        nc.tensor.matmul(t1_ps, lhsT=mg, rhs=xT, start=True, stop=True)
        t1 = work.tile([64, S], F32)
        nc.vector.tensor_copy(out=t1, in_=t1_ps)

        s0_ps = tpsum.tile([64, 1], F32, tag="pt")
        nc.tensor.matmul(s0_ps, lhsT=mg, rhs=ones_col, start=True, stop=True)
        s0 = work.tile([64, 1], F32)
        nc.scalar.copy(out=s0, in_=s0_ps)
        s0r_ps = tpsum.tile([1, C], F32, tag="pt")
        nc.tensor.matmul(s0r_ps, lhsT=ones_col, rhs=mg, start=True, stop=True)
        s0r = work.tile([1, C], F32)
        nc.scalar.copy(out=s0r, in_=s0r_ps)

        # T1 in s-major orientation for the DVE expansion path
        t1s = work.tile([128, 2, C], F32)
        for sh in range(2):
            tp = tpsum.tile([128, C], F32, tag="pt")
            nc.tensor.matmul(tp, lhsT=xT[:, sh * 128:(sh + 1) * 128], rhs=mg,
                             start=True, stop=True)
            nc.vector.tensor_copy(out=t1s[:, sh, :], in_=tp)

        # ---- masks ------------------------------------------------------
        mask = work.tile([64, 1], F32)
        nc.vector.tensor_scalar(out=mask, in0=s0, scalar1=0.0, scalar2=None,
                                op0=ALU.is_gt)
        maskc = work.tile([64, 1], F32)
        nc.vector.tensor_scalar(out=maskc, in0=s0, scalar1=0.0, scalar2=None,
                                op0=ALU.is_le)
        maskc_r = work.tile([1, C], F32)
        nc.vector.tensor_scalar(out=maskc_r, in0=s0r, scalar1=0.0, scalar2=None,
                                op0=ALU.is_le)
        mask_r = work.tile([1, C], F32)
        nc.vector.tensor_scalar(out=mask_r, in0=s0r, scalar1=0.0, scalar2=None,
                                op0=ALU.is_gt)

        # per-head rows: samr[h] = A0[h]*S0 + (1-mask), broadcast to 128
        # partitions via K=1 matmuls (with mask_r in slot 0)
        rows = work.tile([1, 5, C], F32)
        nc.vector.tensor_copy(out=rows[:, 0, :], in_=mask_r)
        for hh in range(NH):
            nc.vector.tensor_scalar_mul(out=rows[:, 1 + hh, :], in0=s0r,
                                        scalar1=float(a0[hh]))
            nc.vector.tensor_add(out=rows[:, 1 + hh, :], in0=rows[:, 1 + hh, :],
                                 in1=maskc_r)
        rowsB_ps = tpsum.tile([128, 5, C], F32, tag="rb")
        nc.tensor.matmul(rowsB_ps, lhsT=ones_row, rhs=rows[:, :, :],
                         start=True, stop=True)
        rowsB = work.tile([128, 5, C], F32)   # [.,0,.]=maskB, [.,1+h,.]=samB
        nc.vector.tensor_copy(out=rowsB, in_=rowsB_ps)

        # ---- r in c-major layout (flatten source for the PE path) ------
        rall = work.tile([64, 5, S], F32)
        nc.vector.tensor_scalar(out=rall[:, 0, :], in0=t1, scalar1=0.0,
                                scalar2=mask, op0=ALU.mult, op1=ALU.add)
        rscr = work.tile([64, S], F32)
        for hh in range(NH):
            sam = work.tile([64, 1], F32)
            nc.vector.tensor_scalar(out=sam, in0=s0, scalar1=float(a0[hh]),
                                    scalar2=maskc, op0=ALU.mult, op1=ALU.add)
            den = work.tile([64, S], F32)
            nc.vector.tensor_scalar(out=den, in0=t1, scalar1=da[hh],
                                    scalar2=sam, op0=ALU.mult, op1=ALU.add)
            rinv = work.tile([64, S], F32)
            nc.vector.reciprocal_approx_accurate(out=rinv, in_=den, scratch=rscr)
            nc.vector.scalar_tensor_tensor(
                out=rall[:, 1 + hh, :], in0=t1, scalar=float(a1[hh]), in1=rinv,
                op0=ALU.mult, op1=ALU.mult,
            )

        rt = consts.tile([5, PE_C, S], F32)
        for j in range(5):
            nc.sync.dma_start(out=rt[j:j + 1, :, :], in_=rall[0:PE_C, j, :])

        # ---- r in s-major layout (scalars for the DVE path) ------------
        rs = work.tile([128, 2, NH, C], F32)
        rscr2 = work.tile([128, C], F32)
        for sh in range(2):
            for hh in range(NH):
                dens = work.tile([128, C], F32)
                nc.vector.tensor_scalar(out=dens, in0=t1s[:, sh, :],
                                        scalar1=da[hh], scalar2=None,
                                        op0=ALU.mult)
                nc.vector.tensor_add(out=dens, in0=dens, in1=rowsB[:, 1 + hh, :])
                rinvs = work.tile([128, C], F32)
                nc.vector.reciprocal_approx_accurate(out=rinvs, in_=dens,
                                                     scratch=rscr2)
                nc.vector.tensor_mul(out=rinvs, in0=rinvs, in1=t1s[:, sh, :])
                nc.vector.tensor_scalar_mul(out=rs[:, sh, hh, :], in0=rinvs,
                                            scalar1=float(a1[hh]))

        # ---- expansion: PE set (pos = (c, s)) --------------------------
        for c in range(PE_C):
            ps = opsum.tile([128, 2, HID], F32, tag="ops")
            for sh in range(2):
                nc.tensor.matmul(ps[:, sh, :],
                                 lhsT=rt[:, c, sh * 128:(sh + 1) * 128],
                                 rhs=waug, start=True, stop=True)
            st = outp.tile([128, 2, HID], F32, tag="st")
            nc.scalar.copy(out=st, in_=ps)
            for sh in range(2):
                nc.sync.dma_start(out=out_d[sh * 128:(sh + 1) * 128, c, :],
                                  in_=st[:, sh, :])

        # ---- expansion: DVE set ----------------------------------------
        for c in range(PE_C, C):
            for sh in range(2):
                st = outp.tile([128, HID], F32, tag="std")
                nc.vector.tensor_scalar_mul(out=st, in0=wf[:, 0:HID],
                                            scalar1=rowsB[:, 0, c:c + 1])
                for hh in range(NH):
                    nc.vector.scalar_tensor_tensor(
                        out=st, in0=wf[:, (1 + hh) * HID:(2 + hh) * HID],
                        scalar=rs[:, sh, hh, c:c + 1], in1=st,
                        op0=ALU.mult, op1=ALU.add,
                    )
                nc.sync.dma_start(out=out_d[sh * 128:(sh + 1) * 128, c, :],
                                  in_=st)

    nc.finalize()
    return nc


_CACHE = {}


def _program(csp, ctp, a0, a1):
    key = (csp, ctp, tuple(a0), tuple(a1), PE_C)
    if key not in _CACHE:
        _CACHE[key] = _build_program(csp, ctp, a0, a1)
    return _CACHE[key]


def kernel(source_ids, source_cluster_labels, in_cluster_spatial_dist,
           in_cluster_temporal_dist, num_clusters,
           c_sp, sp_w1, sp_w2, c_tp, tp_w1, tp_w2,
           attn_w, attn_b, val_w, val_b, ffn_w, ffn_b):
    global LAST_RESULT

    x = np.ascontiguousarray(np.asarray(source_ids), dtype=np.float32)
    lab = np.asarray(source_cluster_labels).astype(np.float32)
    dsp = np.asarray(in_cluster_spatial_dist).astype(np.float32)
    dtp = np.asarray(in_cluster_temporal_dist).astype(np.float32)
    assert int(np.asarray(num_clusters)) == C

    csp = float(max(float(np.asarray(c_sp)), 0.0))
    ctp = float(max(float(np.asarray(c_tp)), 0.0))
    lsp = float(np.log(csp * MAX_SP + 1.0))
    ltp = float(np.log(ctp * MAX_TP + 1.0))

    sp_w1 = np.asarray(sp_w1, dtype=np.float32)   # (FH,1)
    sp_w2 = np.asarray(sp_w2, dtype=np.float32)   # (1,FH)
    tp_w1 = np.asarray(tp_w1, dtype=np.float32)
    tp_w2 = np.asarray(tp_w2, dtype=np.float32)

    cv = np.zeros((1, 192), dtype=np.float32)
    cv[0, 0:64] = np.arange(C, dtype=np.float32)
    cv[0, 64:96] = sp_w1[:, 0] / lsp
    cv[0, 96:128] = tp_w1[:, 0] / ltp
    cv[0, 128:160] = sp_w2[0]
    cv[0, 160:192] = tp_w2[0]

    attn_w = np.asarray(attn_w, dtype=np.float64)
    attn_b = np.asarray(attn_b, dtype=np.float64)
    a0 = np.exp(attn_b)
    a1 = np.exp(attn_b + attn_w)

    val_w = np.asarray(val_w, dtype=np.float64)
    val_b = np.asarray(val_b, dtype=np.float64)
    ffn_w = np.asarray(ffn_w, dtype=np.float64)
    ffn_b = np.asarray(ffn_b, dtype=np.float64)
    waug = np.zeros((5, HID), dtype=np.float64)
    waug[0] = ffn_b + ffn_w @ val_b
    for h in range(NH):
        blk = slice(h * 32, (h + 1) * 32)
        waug[1 + h] = ffn_w[:, blk] @ val_w[blk]
    waug = np.ascontiguousarray(waug, dtype=np.float32)
    wf = np.ascontiguousarray(waug.reshape(1, -1))

    nc = _program(csp, ctp, tuple(a0.tolist()), tuple(a1.tolist()))

    in_maps = []
    for b in range(B):
        nv = np.zeros((N, 4), dtype=np.float32)
        nv[:, 0] = lab[b]
        nv[:, 1] = dsp[b]
        nv[:, 2] = dtp[b]
        in_maps.append({"x": x[b], "nv": nv, "cv": cv, "waug": waug, "wf": wf})

    res = run_bass_kernel_spmd(nc, in_maps, core_ids=list(range(B)),
                               trace=TRACE)
    LAST_RESULT = res
    out = np.stack([res.results[b]["out"] for b in range(B)], axis=0)
    return out


# revision 9
# speedup vs baseline: 1.4407x; 1.4407x over previous
"""Trainium2 Bass kernel for nn_STSourceModule (segment_reduce).

Math: source_ids x are binary {0,1}, so the masked softmax over sites
collapses to a closed form.  With g[n] = exp(fire_bias[n]),
A0[h] = exp(attn_b[h]), A1[h] = exp(attn_b[h] + attn_w[h]):

  Z[s,h,c]   = A0[h]*(S0[c] - T1[s,c]) + A1[h]*T1[s,c]
  r[s,c,h]   = A1[h]*T1[s,c] / Z[s,h,c]
  out[s,c,:] = mask[c]*base + sum_h r[s,c,h]*Wh[h,:]

where S0[c] = sum_{n in c} g[n], T1[s,c] = sum_{n in c} x[s,n]*g[n],
base = ffn_b + ffn_w@val_b, Wh[h] = ffn_w[:,32h:32h+32]@val_w[32h:32h+32].

Sharding: data-parallel over batch B=8, one batch element per core.

The rank-5 expansion runs on the PE as a 3-term fp16 decomposition
(R@W = Rhi@Whi + Rhi@Wlo + Rlo@Whi, dropped term ~2^-22) accumulated in
fp32 PSUM — fp16 operands stream 2x faster than fp32's hi/lo passes.
"""

import sys

for _p in ("/opt/trn_rl_repo",):
    if _p not in sys.path:
        sys.path.insert(0, _p)

from contextlib import ExitStack

import numpy as np

import concourse.bass as bass
import concourse.tile as tile
from concourse import bacc, mybir
from concourse.bass_utils import run_bass_kernel_spmd
from concourse.masks import make_identity

F32 = mybir.dt.float32
F16 = mybir.dt.float16
AF = mybir.ActivationFunctionType
ALU = mybir.AluOpType

MAX_SP, MAX_TP = 180.0, 365.0
B, S, N, C = 8, 256, 128, 64
NH, HID, FH = 4, 256, 32

TRACE = False           # set True (e.g. from test.py) to neuron-profile
LAST_RESULT = None      # BassKernelResults of the last run


def _build_program(csp, ctp, a0, a1):
    nc = bacc.Bacc()

    x_d = nc.declare_dram_parameter("x", [S, N], F32, isOutput=False)
    nv_d = nc.declare_dram_parameter("nv", [N, 4], F32, isOutput=False)
    cv_d = nc.declare_dram_parameter("cv", [1, 192], F32, isOutput=False)
    wh_d = nc.declare_dram_parameter("wh", [5, HID], F16, isOutput=False)
    wl_d = nc.declare_dram_parameter("wl", [5, HID], F16, isOutput=False)
    out_d = nc.declare_dram_parameter("out", [S, C, HID], F32, isOutput=True)

    da = [float(a1[h] - a0[h]) for h in range(NH)]

    with tile.TileContext(nc) as tc, ExitStack() as ctx:
        consts = ctx.enter_context(tc.tile_pool(name="consts", bufs=1))
        work = ctx.enter_context(tc.tile_pool(name="work", bufs=2))
        tpsum = ctx.enter_context(tc.tile_pool(name="tpsum", bufs=2, space="PSUM"))
        opsum = ctx.enter_context(tc.tile_pool(name="opsum", bufs=3, space="PSUM"))
        outp = ctx.enter_context(tc.tile_pool(name="outp", bufs=4))

        # ---- inputs (one DMA each) -------------------------------------
        ident = consts.tile([128, 128], F32)
        make_identity(nc, ident)

        cv = consts.tile([128, 192], F32)
        nc.sync.dma_start(out=cv, in_=cv_d[:, :].to_broadcast([128, 192]))
        wh = consts.tile([5, HID], F16)
        nc.sync.dma_start(out=wh, in_=wh_d[:, :])
        wl = consts.tile([5, HID], F16)
        nc.sync.dma_start(out=wl, in_=wl_d[:, :])
        nv = consts.tile([128, 4], F32)
        nc.sync.dma_start(out=nv, in_=nv_d[:, :])
        xf = consts.tile([128, 2, N], F32)
        nc.sync.dma_start(out=xf, in_=x_d[:, :].rearrange("(q p) n -> p q n", p=128))

        ones_col = consts.tile([128, 1], F32)
        nc.vector.memset(ones_col, 1.0)

        # cv layout: [iota(64) | w1s(32) w1t(32) | w2s(32) w2t(32)]
        iot, w1cat, w2cat = cv[:, 0:64], cv[:, 64:128], cv[:, 128:192]
        labs, dsp, dtp = nv[:, 0:1], nv[:, 1:2], nv[:, 2:3]

        # ---- FIRE bias -> g = exp(bias) --------------------------------
        dls = work.tile([128, 1], F32)
        nc.scalar.activation(out=dls, in_=dsp, func=AF.Ln, bias=1.0, scale=csp)
        dlt = work.tile([128, 1], F32)
        nc.scalar.activation(out=dlt, in_=dtp, func=AF.Ln, bias=1.0, scale=ctp)
        h = work.tile([128, 2 * FH], F32)
        nc.vector.tensor_scalar_mul(out=h[:, 0:FH], in0=w1cat[:, 0:FH], scalar1=dls)
        nc.vector.tensor_scalar_mul(out=h[:, FH:], in0=w1cat[:, FH:], scalar1=dlt)
        nc.scalar.activation(out=h, in_=h, func=AF.Silu)
        nc.vector.tensor_mul(out=h, in0=h, in1=w2cat)
        bsum = work.tile([128, 1], F32)
        nc.vector.reduce_sum(out=bsum, in_=h, axis=mybir.AxisListType.X)
        g = work.tile([128, 1], F32)
        nc.scalar.activation(out=g, in_=bsum, func=AF.Exp)

        # ---- mg[n,c] = (lab[n]==c) * g[n] ------------------------------
        mg = work.tile([128, C], F32)
        nc.vector.tensor_scalar(
            out=mg, in0=iot, scalar1=labs, scalar2=g,
            op0=ALU.is_equal, op1=ALU.mult,
        )

        # ---- xT via PE transpose (dummy first: lone-LDWEIGHTS sync) ----
        ptd = tpsum.tile([128, 128], F32, tag="pt")
        nc.tensor.transpose(ptd, ident, ident)
        xT = work.tile([128, S], F32)
        for i in range(2):
            pt = tpsum.tile([128, 128], F32, tag="pt")
            nc.tensor.transpose(pt, xf[:, i, :], ident)
            nc.scalar.copy(out=xT[:, i * 128:(i + 1) * 128], in_=pt)

        # ---- segment sums ----------------------------------------------
        t1_ps = tpsum.tile([64, S], F32, tag="pt")
        nc.tensor.matmul(t1_ps, lhsT=mg, rhs=xT, start=True, stop=True)
        t1 = work.tile([64, S], F32)
        nc.vector.tensor_copy(out=t1, in_=t1_ps)

        s0_ps = tpsum.tile([64, 1], F32, tag="pt")
        nc.tensor.matmul(s0_ps, lhsT=mg, rhs=ones_col, start=True, stop=True)
        s0 = work.tile([64, 1], F32)
        nc.scalar.copy(out=s0, in_=s0_ps)

        mask = work.tile([64, 1], F32)
        nc.vector.tensor_scalar(out=mask, in0=s0, scalar1=0.0, scalar2=None,
                                op0=ALU.is_gt)
        maskc = work.tile([64, 1], F32)
        nc.vector.tensor_scalar(out=maskc, in0=s0, scalar1=0.0, scalar2=None,
                                op0=ALU.is_le)

        # ---- r planes: rall[c, j, s], j=0 mask, j=1..4 heads -----------
        rall = work.tile([64, 5, S], F32)
        nc.vector.tensor_scalar(out=rall[:, 0, :], in0=t1, scalar1=0.0,
                                scalar2=mask, op0=ALU.mult, op1=ALU.add)
        rscr = work.tile([64, S], F32)
        for hh in range(NH):
            sam = work.tile([64, 1], F32)
            nc.vector.tensor_scalar(out=sam, in0=s0, scalar1=float(a0[hh]),
                                    scalar2=maskc, op0=ALU.mult, op1=ALU.add)
            den = work.tile([64, S], F32)
            nc.vector.tensor_scalar(out=den, in0=t1, scalar1=da[hh],
                                    scalar2=sam, op0=ALU.mult, op1=ALU.add)
            rinv = work.tile([64, S], F32)
            nc.vector.reciprocal_approx_accurate(out=rinv, in_=den, scratch=rscr)
            nc.vector.scalar_tensor_tensor(
                out=rall[:, 1 + hh, :], in0=t1, scalar=float(a1[hh]), in1=rinv,
                op0=ALU.mult, op1=ALU.mult,
            )

        # fp16 hi/lo split of the r planes
        rhi = work.tile([64, 5, S], F16)
        nc.vector.tensor_copy(out=rhi, in_=rall)
        rlo = work.tile([64, 5, S], F16)
        nc.vector.tensor_sub(out=rlo, in0=rall, in1=rhi)

        rth = consts.tile([5, C, S], F16)
        rtl = consts.tile([5, C, S], F16)
        for j in range(5):
            nc.sync.dma_start(out=rth[j:j + 1, :, :], in_=rhi[:, j, :])
            nc.sync.dma_start(out=rtl[j:j + 1, :, :], in_=rlo[:, j, :])

        # ---- expansion: out[.,c,:] = Rt[:,c,.]^T @ W, 3-term fp16 ------
        for cp in range(C // 2):
            ps = opsum.tile([128, 2, 2, HID], F32, tag="ops")  # [p, sh, ci, k]
            for sh in range(2):
                srange = slice(sh * 128, (sh + 1) * 128)
                for ci in range(2):
                    c = cp * 2 + ci
                    dst = ps[:, sh, ci, :]
                    nc.tensor.matmul(dst, lhsT=rth[:, c, srange], rhs=wh,
                                     start=True, stop=False)
                    nc.tensor.matmul(dst, lhsT=rth[:, c, srange], rhs=wl,
                                     start=False, stop=False)
                    nc.tensor.matmul(dst, lhsT=rtl[:, c, srange], rhs=wh,
                                     start=False, stop=True)
            st = outp.tile([128, 2, 2, HID], F32, tag="st")
            nc.scalar.copy(out=st, in_=ps)
            for sh in range(2):
                nc.sync.dma_start(
                    out=out_d[sh * 128:(sh + 1) * 128, cp * 2:cp * 2 + 2, :],
                    in_=st[:, sh, :, :],
                )

    nc.finalize()
    return nc


_CACHE = {}


def _program(csp, ctp, a0, a1):
    key = (csp, ctp, tuple(a0), tuple(a1))
    if key not in _CACHE:
        _CACHE[key] = _build_program(csp, ctp, a0, a1)
    return _CACHE[key]


def kernel(source_ids, source_cluster_labels, in_cluster_spatial_dist,
           in_cluster_temporal_dist, num_clusters,
           c_sp, sp_w1, sp_w2, c_tp, tp_w1, tp_w2,
           attn_w, attn_b, val_w, val_b, ffn_w, ffn_b):
    global LAST_RESULT

    x = np.ascontiguousarray(np.asarray(source_ids), dtype=np.float32)
    lab = np.asarray(source_cluster_labels).astype(np.float32)
    dsp = np.asarray(in_cluster_spatial_dist).astype(np.float32)
    dtp = np.asarray(in_cluster_temporal_dist).astype(np.float32)
    assert int(np.asarray(num_clusters)) == C

    csp = float(max(float(np.asarray(c_sp)), 0.0))
    ctp = float(max(float(np.asarray(c_tp)), 0.0))
    lsp = float(np.log(csp * MAX_SP + 1.0))
    ltp = float(np.log(ctp * MAX_TP + 1.0))

    sp_w1 = np.asarray(sp_w1, dtype=np.float32)   # (FH,1)
    sp_w2 = np.asarray(sp_w2, dtype=np.float32)   # (1,FH)
    tp_w1 = np.asarray(tp_w1, dtype=np.float32)
    tp_w2 = np.asarray(tp_w2, dtype=np.float32)

    cv = np.zeros((1, 192), dtype=np.float32)
    cv[0, 0:64] = np.arange(C, dtype=np.float32)
    cv[0, 64:96] = sp_w1[:, 0] / lsp
    cv[0, 96:128] = tp_w1[:, 0] / ltp
    cv[0, 128:160] = sp_w2[0]
    cv[0, 160:192] = tp_w2[0]

    attn_w = np.asarray(attn_w, dtype=np.float64)
    attn_b = np.asarray(attn_b, dtype=np.float64)
    a0 = np.exp(attn_b)
    a1 = np.exp(attn_b + attn_w)

    val_w = np.asarray(val_w, dtype=np.float64)
    val_b = np.asarray(val_b, dtype=np.float64)
    ffn_w = np.asarray(ffn_w, dtype=np.float64)
    ffn_b = np.asarray(ffn_b, dtype=np.float64)
    waug = np.zeros((5, HID), dtype=np.float64)
    waug[0] = ffn_b + ffn_w @ val_b
    for h in range(NH):
        blk = slice(h * 32, (h + 1) * 32)
        waug[1 + h] = ffn_w[:, blk] @ val_w[blk]
    w_hi = waug.astype(np.float16)
    w_lo = (waug - w_hi.astype(np.float64)).astype(np.float16)

    nc = _program(csp, ctp, tuple(a0.tolist()), tuple(a1.tolist()))

    in_maps = []
    for b in range(B):
        nv = np.zeros((N, 4), dtype=np.float32)
        nv[:, 0] = lab[b]
        nv[:, 1] = dsp[b]
        nv[:, 2] = dtp[b]
        in_maps.append({"x": x[b], "nv": nv, "cv": cv,
                        "wh": np.ascontiguousarray(w_hi),
                        "wl": np.ascontiguousarray(w_lo)})

    res = run_bass_kernel_spmd(nc, in_maps, core_ids=list(range(B)),
                               trace=TRACE)
    LAST_RESULT = res
    out = np.stack([res.results[b]["out"] for b in range(B)], axis=0)
    return out


# revision 11
# speedup vs baseline: 1.9929x; 1.3833x over previous
"""Trainium2 Bass kernel for nn_STSourceModule (segment_reduce).

Math: source_ids x are binary {0,1}, so the masked softmax over sites
collapses to a closed form.  With g[n] = exp(fire_bias[n]),
A0[h] = exp(attn_b[h]), A1[h] = exp(attn_b[h] + attn_w[h]):

  Z[s,h,c]   = A0[h]*(S0[c] - T1[s,c]) + A1[h]*T1[s,c]
  r[s,c,h]   = A1[h]*T1[s,c] / Z[s,h,c]
  out[s,c,:] = mask[c]*base + sum_h r[s,c,h]*Wh[h,:]

where S0[c] = sum_{n in c} g[n], T1[s,c] = sum_{n in c} x[s,n]*g[n],
base = ffn_b + ffn_w@val_b, Wh[h] = ffn_w[:,32h:32h+32]@val_w[32h:32h+32].

Sharding: data-parallel over batch B=8, one batch element per core.

Expansion trick: R@W in fp16-pair precision as a SINGLE K=20 matmul per
tile — lhsT stacks [Rhi;Rlo;Rhi;Rlo] (fp16) and rhs stacks
[Wh;Wl;Wl;Wh], so the K-contraction sums all four partial products
(Rhi@Wh + Rlo@Wl + Rhi@Wl + Rlo@Wh = exact pair product) in fp32 PSUM
at the PE cost of one 256-column fp16 pass (~214ns/tile-pass).
"""

import sys

for _p in ("/opt/trn_rl_repo",):
    if _p not in sys.path:
        sys.path.insert(0, _p)

from contextlib import ExitStack

import numpy as np

import concourse.bass as bass
import concourse.tile as tile
from concourse import bacc, mybir
from concourse.bass_utils import run_bass_kernel_spmd
from concourse.masks import make_identity

F32 = mybir.dt.float32
F16 = mybir.dt.float16
AF = mybir.ActivationFunctionType
ALU = mybir.AluOpType

MAX_SP, MAX_TP = 180.0, 365.0
B, S, N, C = 8, 256, 128, 64
NH, HID, FH = 4, 256, 32

TRACE = False           # set True (e.g. from test.py) to neuron-profile
LAST_RESULT = None      # BassKernelResults of the last run


def _build_program(csp, ctp, a0, a1):
    nc = bacc.Bacc()

    x_d = nc.declare_dram_parameter("x", [S, N], F32, isOutput=False)
    nv_d = nc.declare_dram_parameter("nv", [N, 4], F32, isOutput=False)
    cv_d = nc.declare_dram_parameter("cv", [1, 192], F32, isOutput=False)
    wq_d = nc.declare_dram_parameter("wq", [20, HID], F16, isOutput=False)
    out_d = nc.declare_dram_parameter("out", [S, C, HID], F32, isOutput=True)

    da = [float(a1[h] - a0[h]) for h in range(NH)]

    with tile.TileContext(nc) as tc, ExitStack() as ctx:
        consts = ctx.enter_context(tc.tile_pool(name="consts", bufs=1))
        work = ctx.enter_context(tc.tile_pool(name="work", bufs=2))
        tpsum = ctx.enter_context(tc.tile_pool(name="tpsum", bufs=2, space="PSUM"))
        opsum = ctx.enter_context(tc.tile_pool(name="opsum", bufs=3, space="PSUM"))
        outp = ctx.enter_context(tc.tile_pool(name="outp", bufs=4))

        # ---- inputs (one DMA each) -------------------------------------
        ident = consts.tile([128, 128], F32)
        make_identity(nc, ident)

        cv = consts.tile([128, 192], F32)
        nc.sync.dma_start(out=cv, in_=cv_d[:, :].to_broadcast([128, 192]))
        wq = consts.tile([20, HID], F16)
        nc.sync.dma_start(out=wq, in_=wq_d[:, :])
        nv = consts.tile([128, 4], F32)
        nc.sync.dma_start(out=nv, in_=nv_d[:, :])
        xf = consts.tile([128, 2, N], F32)
        nc.sync.dma_start(out=xf, in_=x_d[:, :].rearrange("(q p) n -> p q n", p=128))

        ones_col = consts.tile([128, 1], F32)
        nc.vector.memset(ones_col, 1.0)

        # cv layout: [iota(64) | w1s(32) w1t(32) | w2s(32) w2t(32)]
        iot, w1cat, w2cat = cv[:, 0:64], cv[:, 64:128], cv[:, 128:192]
        labs, dsp, dtp = nv[:, 0:1], nv[:, 1:2], nv[:, 2:3]

        # ---- FIRE bias -> g = exp(bias) --------------------------------
        dls = work.tile([128, 1], F32)
        nc.scalar.activation(out=dls, in_=dsp, func=AF.Ln, bias=1.0, scale=csp)
        dlt = work.tile([128, 1], F32)
        nc.scalar.activation(out=dlt, in_=dtp, func=AF.Ln, bias=1.0, scale=ctp)
        h = work.tile([128, 2 * FH], F32)
        nc.vector.tensor_scalar_mul(out=h[:, 0:FH], in0=w1cat[:, 0:FH], scalar1=dls)
        nc.vector.tensor_scalar_mul(out=h[:, FH:], in0=w1cat[:, FH:], scalar1=dlt)
        nc.scalar.activation(out=h, in_=h, func=AF.Silu)
        nc.vector.tensor_mul(out=h, in0=h, in1=w2cat)
        bsum = work.tile([128, 1], F32)
        nc.vector.reduce_sum(out=bsum, in_=h, axis=mybir.AxisListType.X)
        g = work.tile([128, 1], F32)
        nc.scalar.activation(out=g, in_=bsum, func=AF.Exp)

        # ---- mg[n,c] = (lab[n]==c) * g[n] ------------------------------
        mg = work.tile([128, C], F32)
        nc.vector.tensor_scalar(
            out=mg, in0=iot, scalar1=labs, scalar2=g,
            op0=ALU.is_equal, op1=ALU.mult,
        )

        # ---- xT via PE transpose (dummy first: lone-LDWEIGHTS sync) ----
        ptd = tpsum.tile([128, 128], F32, tag="pt")
        nc.tensor.transpose(ptd, ident, ident)
        xT = work.tile([128, S], F32)
        for i in range(2):
            pt = tpsum.tile([128, 128], F32, tag="pt")
            nc.tensor.transpose(pt, xf[:, i, :], ident)
            nc.scalar.copy(out=xT[:, i * 128:(i + 1) * 128], in_=pt)

        # ---- segment sums ----------------------------------------------
        t1_ps = tpsum.tile([64, S], F32, tag="pt")
        nc.tensor.matmul(t1_ps, lhsT=mg, rhs=xT, start=True, stop=True)
        t1 = work.tile([64, S], F32)
        nc.vector.tensor_copy(out=t1, in_=t1_ps)

        s0_ps = tpsum.tile([64, 1], F32, tag="pt")
        nc.tensor.matmul(s0_ps, lhsT=mg, rhs=ones_col, start=True, stop=True)
        s0 = work.tile([64, 1], F32)
        nc.scalar.copy(out=s0, in_=s0_ps)

        mask = work.tile([64, 1], F32)
        nc.vector.tensor_scalar(out=mask, in0=s0, scalar1=0.0, scalar2=None,
                                op0=ALU.is_gt)
        maskc = work.tile([64, 1], F32)
        nc.vector.tensor_scalar(out=maskc, in0=s0, scalar1=0.0, scalar2=None,
                                op0=ALU.is_le)

        # ---- r planes + fp16 hi/lo split + flatten, pipelined per j ----
        # rt20 partitions: [0:5]=Rhi, [5:10]=Rlo, [10:15]=Rhi, [15:20]=Rlo
        rt20 = consts.tile([20, C, S], F16)
        rall = work.tile([64, 5, S], F32)
        rscr = work.tile([64, S], F32)

        def split_and_flatten(j):
            hi = work.tile([64, S], F16, tag="hi16")
            nc.vector.tensor_copy(out=hi, in_=rall[:, j, :])
            lo = work.tile([64, S], F16, tag="lo16")
            nc.vector.tensor_sub(out=lo, in0=rall[:, j, :], in1=hi)
            nc.sync.dma_start(out=rt20[j:j + 1, :, :], in_=hi)
            nc.sync.dma_start(out=rt20[10 + j:11 + j, :, :], in_=hi)
            nc.sync.dma_start(out=rt20[5 + j:6 + j, :, :], in_=lo)
            nc.sync.dma_start(out=rt20[15 + j:16 + j, :, :], in_=lo)

        nc.vector.tensor_scalar(out=rall[:, 0, :], in0=t1, scalar1=0.0,
                                scalar2=mask, op0=ALU.mult, op1=ALU.add)
        split_and_flatten(0)
        for hh in range(NH):
            sam = work.tile([64, 1], F32)
            nc.vector.tensor_scalar(out=sam, in0=s0, scalar1=float(a0[hh]),
                                    scalar2=maskc, op0=ALU.mult, op1=ALU.add)
            den = work.tile([64, S], F32)
            nc.vector.tensor_scalar(out=den, in0=t1, scalar1=da[hh],
                                    scalar2=sam, op0=ALU.mult, op1=ALU.add)
            rinv = work.tile([64, S], F32)
            nc.vector.reciprocal_approx_accurate(out=rinv, in_=den, scratch=rscr)
            nc.vector.scalar_tensor_tensor(
                out=rall[:, 1 + hh, :], in0=t1, scalar=float(a1[hh]), in1=rinv,
                op0=ALU.mult, op1=ALU.mult,
            )
            split_and_flatten(1 + hh)

        # ---- expansion: one K=20 fp16 matmul per (c, s-half) -----------
        for cp in range(C // 2):
            ps = opsum.tile([128, 2, 2, HID], F32, tag="ops")  # [p, sh, ci, k]
            for sh in range(2):
                srange = slice(sh * 128, (sh + 1) * 128)
                for ci in range(2):
                    nc.tensor.matmul(ps[:, sh, ci, :],
                                     lhsT=rt20[:, cp * 2 + ci, srange],
                                     rhs=wq, start=True, stop=True)
            st = outp.tile([128, 2, 2, HID], F32, tag="st")
            nc.scalar.copy(out=st, in_=ps)
            for sh in range(2):
                nc.sync.dma_start(
                    out=out_d[sh * 128:(sh + 1) * 128, cp * 2:cp * 2 + 2, :],
                    in_=st[:, sh, :, :],
                )

    nc.finalize()
    return nc


_CACHE = {}


def _program(csp, ctp, a0, a1):
    key = (csp, ctp, tuple(a0), tuple(a1))
    if key not in _CACHE:
        _CACHE[key] = _build_program(csp, ctp, a0, a1)
    return _CACHE[key]


def kernel(source_ids, source_cluster_labels, in_cluster_spatial_dist,
           in_cluster_temporal_dist, num_clusters,
           c_sp, sp_w1, sp_w2, c_tp, tp_w1, tp_w2,
           attn_w, attn_b, val_w, val_b, ffn_w, ffn_b):
    global LAST_RESULT

    x = np.ascontiguousarray(np.asarray(source_ids), dtype=np.float32)
    lab = np.asarray(source_cluster_labels).astype(np.float32)
    dsp = np.asarray(in_cluster_spatial_dist).astype(np.float32)
    dtp = np.asarray(in_cluster_temporal_dist).astype(np.float32)
    assert int(np.asarray(num_clusters)) == C

    csp = float(max(float(np.asarray(c_sp)), 0.0))
    ctp = float(max(float(np.asarray(c_tp)), 0.0))
    lsp = float(np.log(csp * MAX_SP + 1.0))
    ltp = float(np.log(ctp * MAX_TP + 1.0))

    sp_w1 = np.asarray(sp_w1, dtype=np.float32)   # (FH,1)
    sp_w2 = np.asarray(sp_w2, dtype=np.float32)   # (1,FH)
    tp_w1 = np.asarray(tp_w1, dtype=np.float32)
    tp_w2 = np.asarray(tp_w2, dtype=np.float32)

    cv = np.zeros((1, 192), dtype=np.float32)
    cv[0, 0:64] = np.arange(C, dtype=np.float32)
    cv[0, 64:96] = sp_w1[:, 0] / lsp
    cv[0, 96:128] = tp_w1[:, 0] / ltp
    cv[0, 128:160] = sp_w2[0]
    cv[0, 160:192] = tp_w2[0]

    attn_w = np.asarray(attn_w, dtype=np.float64)
    attn_b = np.asarray(attn_b, dtype=np.float64)
    a0 = np.exp(attn_b)
    a1 = np.exp(attn_b + attn_w)

    val_w = np.asarray(val_w, dtype=np.float64)
    val_b = np.asarray(val_b, dtype=np.float64)
    ffn_w = np.asarray(ffn_w, dtype=np.float64)
    ffn_b = np.asarray(ffn_b, dtype=np.float64)
    waug = np.zeros((5, HID), dtype=np.float64)
    waug[0] = ffn_b + ffn_w @ val_b
    for h in range(NH):
        blk = slice(h * 32, (h + 1) * 32)
        waug[1 + h] = ffn_w[:, blk] @ val_w[blk]
    w_hi = waug.astype(np.float16)
    w_lo = (waug - w_hi.astype(np.float64)).astype(np.float16)
    # rhs stack matching lhsT [Rhi;Rlo;Rhi;Rlo]:
    # Rhi@Wh + Rlo@Wl + Rhi@Wl + Rlo@Wh = exact pair product
    wquad = np.ascontiguousarray(
        np.concatenate([w_hi, w_lo, w_lo, w_hi], axis=0))

    nc = _program(csp, ctp, tuple(a0.tolist()), tuple(a1.tolist()))

    in_maps = []
    for b in range(B):
        nv = np.zeros((N, 4), dtype=np.float32)
        nv[:, 0] = lab[b]
        nv[:, 1] = dsp[b]
        nv[:, 2] = dtp[b]
        in_maps.append({"x": x[b], "nv": nv, "cv": cv, "wq": wquad})

    res = run_bass_kernel_spmd(nc, in_maps, core_ids=list(range(B)),
                               trace=TRACE)
    LAST_RESULT = res
    out = np.stack([res.results[b]["out"] for b in range(B)], axis=0)
    return out
